# revision 42
# baseline (speedup 1.0000x reference)
"""Trainium2 Bass kernel for an ALBERT-style seq2seq block (self-attn + cross-attn).

Sharding: 8 cores = (batch b in 0..3) x (decoder-row half in 0..1); zero
inter-core communication. Each core computes its 512 decoder rows.

v4 design vs v2 baseline:
- PV matmuls in fp8 DoubleRow: exp writes probs straight to fp8 slot
  buffers (p8A/p8B), vi (values+ones) is fp8, so each PV pass contracts
  two 128-key blocks -> half the PV passes. The denominator rides the
  same quantized probs, so softmax stays exactly normalized.
- DMA plumbing rebuilt for packet efficiency: host pre-transposes wd and
  wv so every transfer moves >=2KB contiguous per partition; x8/x8e are
  striped across all three DMA-capable queues; the six per-column
  parameter vectors ride one combined tensor; big SBUF memsets moved to
  the (idle at start) Vector engine so the GpSimd queue can issue its
  DMAs immediately.
- Attention starts as early as possible: phase 1 is just k0+q0; k1/q1
  and the v-projection half ride a prefill slot inside attention, after
  the first two score blocks are in flight.
- LayerNorm tails: residual+bias pre-folded (rqb / partialA) so the
  out-proj eviction is a single STT op; x^2 stats on the otherwise-idle
  Scalar engine; rsqrt = Sqrt + fast-reciprocal with the sqrt activation
  table pre-warmed by a dummy op so no table load lands on the critical
  chain; per-o-tile apply pipelined across Vector/GpSimd/Scalar; bf16
  output store (host casts to f32).
- Scalar stays exp-only during attention (q-bias eviction on Vector).
"""

import sys

sys.path.insert(0, "/opt/trn_rl_repo")

import numpy as np
import ml_dtypes

import concourse.bacc as bacc
import concourse.mybir as mybir
from concourse.bass_utils import run_bass_kernel_spmd
from concourse.tile import TileContext

F32 = mybir.dt.float32
F32R = mybir.dt.float32r
BF16 = mybir.dt.bfloat16
F8 = mybir.dt.float8e4
AF = mybir.ActivationFunctionType
ALU = mybir.AluOpType
DR = mybir.MatmulPerfMode.DoubleRow

P = 128          # partitions
H = 1024         # hidden
NT = H // P      # 8 tiles over hidden
NH = 16          # heads
D = 64           # head dim
T = 1024         # sequence length (encoder and decoder)
R = 512          # decoder rows per core
B = 4
EPS = 1e-12
WS = 32.0        # fp8 weight pre-scale (host); descaled at PSUM eviction
INV = 1.0 / WS

# prm rows: combined per-partition column parameters
I_BQ, I_BDE, I_G, I_B, I_MT, I_MS = range(6)


def build_kernel():
    nc = bacc.Bacc("TRN2", num_devices=8)

    dec8_d = nc.declare_dram_parameter("dec8", [P, NT, T], F8, isOutput=False)
    enc8_d = nc.declare_dram_parameter("enc8", [P, NT, T], F8, isOutput=False)
    dqb_d = nc.declare_dram_parameter("dqb", [P, NT, R], BF16, isOutput=False)
    wq_d = nc.declare_dram_parameter("wq", [NT, P, NT, P], BF16, isOutput=False)
    wk_d = nc.declare_dram_parameter("wk", [P, NT, NT, P], F8, isOutput=False)
    wv_d = nc.declare_dram_parameter("wv", [P, 2, NT, R], F8, isOutput=False)
    wd_d = nc.declare_dram_parameter("wd", [P, NT, NT, P], F8, isOutput=False)
    prm_d = nc.declare_dram_parameter("prm", [P, 6, NT, 1], F32, isOutput=False)
    onesrr_d = nc.declare_dram_parameter("onesr", [1, P], F32, isOutput=False)
    out_d = nc.declare_dram_parameter("out", [P, NT, R], BF16, isOutput=True)

    with TileContext(nc) as tc:
        with tc.tile_pool(name="base", bufs=1) as base:
            x8 = base.tile([P, NT, T], F8, tag="x8")
            x8e = base.tile([P, NT, T], F8, tag="x8e")
            dqb = base.tile([P, NT, R], BF16, tag="dqb")
            wk = base.tile([P, NT, NT, P], F8, tag="wk")
            wv = base.tile([P, 2, NT, R], F8, tag="wv")
            wdr = base.tile([P, NT, NT, P], F8, tag="wdr")
            kT = base.tile([P, NT, T], BF16, tag="kT")
            vi = base.tile([P, NT, NH, P], F8, tag="vi")
            q1b = [base.tile([P, R], BF16, tag=f"q1b{o}", name=f"q1b{o}") for o in range(NT)]
            rqb = [base.tile([P, R], BF16, tag=f"rqb{o}", name=f"rqb{o}") for o in range(NT)]
            slfb = [base.tile([P, R], BF16, tag=f"slfb{o}", name=f"slfb{o}") for o in range(NT)]
            p8A = base.tile([P, NT, 2, R], F8, tag="p8A")
            p8B = base.tile([P, NT, 2, R], F8, tag="p8B")
            ctxn = base.tile([P, NT, R], F8, tag="ctxn")
            resT = base.tile([P, NT, R], BF16, tag="resT")
            partialA = base.tile([P, NT, R], BF16, tag="partialA")
            prm = base.tile([P, 6, NT, 1], F32, tag="prm")
            onesb = base.tile([P, 1], BF16, tag="onesb")
            onesr = base.tile([1, P], F32R, tag="onesr")
            warm = base.tile([1, 2], F32, tag="warm")
            epsr = base.tile([1, 1], F32, tag="epsr")

            wuM = base.tile([P, R], BF16, tag="wuM")
            wq0t = base.tile([P, NT, P], BF16, tag="wq0t")
            wq1t = base.tile([P, NT, P], BF16, tag="wq1t")

            # ---- startup DMAs: striped across the 3 DMA-capable queues,
            # ordered by first use ----
            nc.sync.dma_start(out=x8[:, 0:3, :], in_=dec8_d.ap()[:, 0:3, :])
            nc.scalar.dma_start(out=x8[:, 3:6, :], in_=dec8_d.ap()[:, 3:6, :])
            nc.gpsimd.dma_start(out=x8[:, 6:NT, :], in_=dec8_d.ap()[:, 6:NT, :])
            nc.gpsimd.dma_start(out=wk[:, 0:2, :, :], in_=wk_d.ap()[:, 0:2, :, :])
            nc.scalar.dma_start(out=prm[:, :, :, :], in_=prm_d.ap())
            nc.scalar.dma_start(out=dqb[:, 0:2, :], in_=dqb_d.ap()[:, 0:2, :])
            nc.sync.dma_start(out=dqb[:, 4:6, :], in_=dqb_d.ap()[:, 4:6, :])
            nc.scalar.dma_start(out=dqb[:, 2:4, :], in_=dqb_d.ap()[:, 2:4, :])
            nc.sync.dma_start(out=dqb[:, 6:NT, :], in_=dqb_d.ap()[:, 6:NT, :])
            nc.gpsimd.dma_start(out=wq0t[:, :, :], in_=wq_d.ap()[0])
            nc.gpsimd.dma_start(out=wq1t[:, :, :], in_=wq_d.ap()[1])
            nc.sync.dma_start(out=wv[:, 0, :, :], in_=wv_d.ap()[:, 0, :, :])
            nc.sync.dma_start(out=wv[:, 1, :, :], in_=wv_d.ap()[:, 1, :, :])
            nc.gpsimd.dma_start(out=wk[:, 2:NT, :, :], in_=wk_d.ap()[:, 2:NT, :, :])
            nc.scalar.dma_start(out=onesr[:, :], in_=onesrr_d.ap().bitcast(F32R))
            nc.sync.dma_start(out=x8e[:, 0:3, :], in_=enc8_d.ap()[:, 0:3, :])
            nc.scalar.dma_start(out=x8e[:, 3:6, :], in_=enc8_d.ap()[:, 3:6, :])
            nc.gpsimd.dma_start(out=x8e[:, 6:NT, :], in_=enc8_d.ap()[:, 6:NT, :])
            nc.scalar.dma_start(out=wdr[:, :, :, :], in_=wd_d.ap())
            nc.gpsimd.memset(onesb[:, :], 1.0)
            # big memsets on the idle-at-start Vector engine; tiny warm-up
            # tiles first so the exp table load fires immediately
            nc.vector.memset(warm[:, :], 1.0)
            nc.vector.memset(epsr[:, :], EPS)
            nc.vector.memset(wuM[:, :], 0.0)
            nc.scalar.activation(warm[:, 0:1], warm[:, 1:2], AF.Exp)
            for st in range(NT):
                nc.vector.memset(vi[:, st, :, D:P], 1.0)

            rcp = tc.alloc_tile_pool(name="rcp", bufs=8)

            def k_unit(src, ot, ps, uid):
                """One o-tile of a k projection: fp8 DoubleRow matmuls,
                descaled eviction into kT (no k bias needed)."""
                for tch in range(2):
                    tsl = slice(tch * R, (tch + 1) * R)
                    pk = ps.tile([P, R], F32, tag="pk", name=f"pk{uid}_{ot}_{tch}")
                    for i in range(4):
                        nc.tensor.matmul(
                            pk[:, :], wk[:, ot, 2 * i:2 * i + 2, :],
                            src[:, 2 * i:2 * i + 2, tsl],
                            start=(i == 0), stop=(i == 3), perf_mode=DR)
                    nc.vector.tensor_scalar_mul(kT[:, ot, tsl], pk[:, :], INV)

            def v_unit(src, kb, hf, ps, uid):
                """One (key-block, head-half) of a v projection, computed
                directly in [keys, vdim] layout, evicted descaled to fp8."""
                ksl = slice(kb * P, (kb + 1) * P)
                pv = ps.tile([P, R], F32, tag="pk", name=f"pv{uid}_{kb}_{hf}")
                for i in range(4):
                    nc.tensor.matmul(
                        pv[:, :], src[:, 2 * i:2 * i + 2, ksl],
                        wv[:, hf, 2 * i:2 * i + 2, :],
                        start=(i == 0), stop=(i == 3), perf_mode=DR)
                nc.vector.tensor_scalar_mul(
                    vi[:, kb, hf * 8:(hf + 1) * 8, 0:D],
                    pv[:, :].rearrange("p (h c) -> p h c", c=D), INV)

            def q_unit(ot, ps, wp, pre=None):
                """One o-tile of the q projection (bf16, biased); bias
                applied on Vector so Scalar stays exp-only."""
                if pre is None:
                    wqc = wp.tile([P, NT, P], BF16, tag="wqc", name=f"wqc_{ot}")
                    nc.gpsimd.dma_start(out=wqc[:, :, :], in_=wq_d.ap()[ot])
                else:
                    wqc = pre
                pq = ps.tile([P, R], F32, tag="pk", name=f"pq_{ot}")
                for it in range(NT):
                    nc.tensor.matmul(
                        pq[:, :], wqc[:, it, :], dqb[:, it, :],
                        start=(it == 0), stop=(it == NT - 1))
                nc.vector.tensor_scalar(
                    q1b[ot][:, :], pq[:, :], 1.0, prm[:, I_BQ, ot, :],
                    op0=ALU.mult, op1=ALU.add)

            def attention(qsrc, mi, fillers, qsched, uid, p8, prefills=()):
                """scoresT -> exp(fp8 slots) -> fp8-DR PV + fused
                denominator -> staged reciprocal -> ctxn.
                prefills run after the first two score blocks are queued.
                qsched[j][k]: cumulative filler-drain count reached right
                BEFORE the PV matmuls of step (j, k) — so a fill drained at
                (j, k) may feed that very PV step. Fill order must respect
                data readiness: the PE queue is in-order."""
                fill_i = 0
                with tc.tile_pool(name="psc", bufs=2, space="PSUM") as psc, \
                     tc.tile_pool(name="pcx", bufs=1, space="PSUM") as pcx:

                    def emit_scores(idx):
                        j2, st2 = divmod(idx, NT)
                        ssl = slice(st2 * P, (st2 + 1) * P)
                        s01 = psc.tile([P, 2, R], F32, tag="s01", name=f"s{uid}_{j2}_{st2}")
                        nc.tensor.matmul(
                            s01[:, 0, :], kT[0:D, j2, ssl], qsrc[j2][0:D, :])
                        nc.tensor.matmul(
                            s01[:, 1, :], kT[D:P, j2, ssl], qsrc[j2][D:P, :])
                        nc.scalar.activation(
                            p8[:, st2, :, :], s01[:, :, :], AF.Exp,
                            bias=prm[:, mi, st2, :], scale=0.125)

                    # exp runs 2 key-blocks ahead of PV across pair
                    # boundaries so the exp stream never stalls
                    emit_scores(0)
                    emit_scores(1)
                    for pf in prefills:
                        pf()
                    for j in range(NH // 2):
                        c0 = pcx.tile([P, R], F32, tag="c0", name=f"c0{uid}_{j}")
                        c1 = pcx.tile([P, R], F32, tag="c1", name=f"c1{uid}_{j}")
                        for k in range(4):
                            for nxt in (j * NT + 2 * k + 2, j * NT + 2 * k + 3):
                                if nxt < NT * (NH // 2):
                                    emit_scores(nxt)
                            while fill_i < qsched[j][k] and fill_i < len(fillers):
                                fillers[fill_i]()
                                fill_i += 1
                            nc.tensor.matmul(
                                c0[:, :], vi[:, 2 * k:2 * k + 2, 2 * j, :],
                                p8[:, 2 * k:2 * k + 2, 0, :],
                                start=(k == 0), stop=(k == 3), perf_mode=DR)
                            nc.tensor.matmul(
                                c1[:, :], vi[:, 2 * k:2 * k + 2, 2 * j + 1, :],
                                p8[:, 2 * k:2 * k + 2, 1, :],
                                start=(k == 0), stop=(k == 3), perf_mode=DR)
                        d0 = rcp.tile([D, R], F32, tag="rr", bufs=8, name=f"d0{uid}_{j}")
                        r0 = rcp.tile([D, R], F32, tag="rr", bufs=8, name=f"r0{uid}_{j}")
                        d1 = rcp.tile([D, R], F32, tag="rr", bufs=8, name=f"d1{uid}_{j}")
                        r1 = rcp.tile([D, R], F32, tag="rr", bufs=8, name=f"r1{uid}_{j}")
                        # custom-DVE reciprocal needs an SBUF operand; one
                        # staging copy of the denominator rows only
                        nc.vector.tensor_copy(d0[:, :], c0[D:P, :])
                        nc.vector.reciprocal_approx_fast(r0[:, :], d0[:, :])
                        nc.vector.tensor_mul(ctxn[0:D, j, :], c0[0:D, :], r0[:, :])
                        nc.vector.tensor_copy(d1[:, :], c1[D:P, :])
                        nc.vector.reciprocal_approx_fast(r1[:, :], d1[:, :])
                        nc.vector.tensor_mul(ctxn[D:P, j, :], c1[0:D, :], r1[:, :])
                    while fill_i < len(fillers):
                        fillers[fill_i]()
                        fill_i += 1

            def proj_ln(resid_b, dst, fillers, uid, partial=None, hold=0,
                        store=False):
                """Out-projection + pre-folded residual (single STT evict)
                into resT with LN stats fused per o-tile (x^2 on Scalar);
                then row-level stats, rsqrt = Sqrt + fast reciprocal with
                the sqrt table pre-warmed, broadcast, pipelined apply."""
                fill_i = 0
                with tc.tile_pool(name="sqp", bufs=2) as sqp, \
                     tc.tile_pool(name="lnp", bufs=1) as lnp, \
                     tc.tile_pool(name="outp", bufs=2) as outp, \
                     tc.tile_pool(name="ps3", bufs=3, space="PSUM") as ps, \
                     tc.tile_pool(name="ps4", bufs=1, space="PSUM") as ps4:
                    # dummy touch: pulls the sqrt-table load off the stats
                    # chain into the (Scalar-idle) out-proj phase
                    nc.scalar.activation(warm[:, 0:1], warm[:, 1:2], AF.Sqrt)
                    pmu = ps4.tile([1, R], F32, tag="pmu", name=f"pmu{uid}")
                    psq = ps4.tile([1, R], F32, tag="psq", name=f"psq{uid}")
                    if partial is None:
                        # fold bias+residual once (GpSimd, off critical path)
                        for ot in range(NT):
                            nc.gpsimd.tensor_scalar(
                                rqb[ot][:, :], resid_b[ot][:, :], 1.0,
                                prm[:, I_BDE, ot, :], op0=ALU.mult, op1=ALU.add)
                    if store:
                        # tail path: emit all out-proj matmuls + evicts as a
                        # dense PE burst first, then squares+stats stream at
                        # Scalar pace (avoids mid-p-state stats matmuls)
                        for ot in range(NT):
                            pp = ps.tile([P, R], F32, tag="pp", name=f"pp{uid}_{ot}")
                            nc.tensor.matmul(
                                pp[:, :], wdr[:, ot, 6:8, :],
                                ctxn[:, 6:8, :],
                                start=True, stop=True, perf_mode=DR)
                            nc.vector.scalar_tensor_tensor(
                                resT[:, ot, :], pp[:, :], INV,
                                partial[:, ot, :], op0=ALU.mult, op1=ALU.add)
                        for ot in range(NT):
                            sq = sqp.tile([P, R], BF16, tag="sq",
                                          name=f"sq{uid}_{ot}", bufs=3)
                            nc.scalar.square(sq[:, :], resT[:, ot, :])
                            nc.tensor.matmul(
                                pmu[:, :], onesb[:, :], resT[:, ot, :],
                                start=(ot == 0), stop=(ot == NT - 1))
                            nc.tensor.matmul(
                                psq[:, :], onesb[:, :], sq[:, :],
                                start=(ot == 0), stop=(ot == NT - 1))
                    else:
                        for ot in range(NT):
                            pp = ps.tile([P, R], F32, tag="pp", name=f"pp{uid}_{ot}")
                            for i in range(4):
                                nc.tensor.matmul(
                                    pp[:, :], wdr[:, ot, 2 * i:2 * i + 2, :],
                                    ctxn[:, 2 * i:2 * i + 2, :],
                                    start=(i == 0), stop=(i == 3), perf_mode=DR)
                            nc.vector.scalar_tensor_tensor(
                                resT[:, ot, :], pp[:, :], INV,
                                rqb[ot][:, :], op0=ALU.mult, op1=ALU.add)
                            sq = sqp.tile([P, R], BF16, tag="sq",
                                          name=f"sq{uid}_{ot}", bufs=3)
                            nc.scalar.square(sq[:, :], resT[:, ot, :])
                            nc.tensor.matmul(
                                pmu[:, :], onesb[:, :], resT[:, ot, :],
                                start=(ot == 0), stop=(ot == NT - 1))
                            nc.tensor.matmul(
                                psq[:, :], onesb[:, :], sq[:, :],
                                start=(ot == 0), stop=(ot == NT - 1))
                            early = len(fillers) - hold
                            while fillers and fill_i < (ot + 1) * early // NT:
                                fillers[fill_i]()
                                fill_i += 1
                    # held fillers drain now: independent PE work queued
                    # ahead of the broadcast matmuls on the stats chain
                    while fill_i < len(fillers):
                        fillers[fill_i]()
                        fill_i += 1
                    # row-level stats: mu & mu^2 on Scalar, var on Vector,
                    # sd = sqrt(var+eps) (warm table)
                    mu_r = lnp.tile([1, R], F32R, tag="lnrow", bufs=2, name=f"mu{uid}")
                    nc.scalar.mul(mu_r[:, :], pmu[:, :], 1.0 / H)
                    msq_r = lnp.tile([1, R], F32, tag="lnrow", bufs=2, name=f"msq{uid}")
                    nc.scalar.activation(
                        msq_r[:, :], pmu[:, :], AF.Square, scale=1.0 / H)
                    var_r = lnp.tile([1, R], F32, tag="lnrow2", bufs=2, name=f"var{uid}")
                    nc.vector.scalar_tensor_tensor(
                        var_r[:, :], psq[:, :], 1.0 / H, msq_r[:, :],
                        op0=ALU.mult, op1=ALU.subtract)
                    sd_r = lnp.tile([1, R], F32, tag="lnrow2", bufs=2, name=f"sd{uid}")
                    nc.scalar.activation(sd_r[:, :], var_r[:, :], AF.Sqrt,
                                         bias=epsr[:, :])
                    if not store:
                        # pre-warm the exp table for the next attention NOW:
                        # the remaining Scalar ops (Copy) live in every table
                        nc.scalar.activation(warm[:, 0:1], warm[:, 1:2], AF.Exp)
                    rs_r = lnp.tile([1, R], F32, tag="lnrow3", bufs=1, name=f"rs{uid}")
                    nc.vector.reciprocal_approx_fast(rs_r[:, :], sd_r[:, :])
                    # FP32R matmul operands must come from a rounding
                    # producer; bounce through a Scalar copy (table-free)
                    rs_r2 = lnp.tile([1, R], F32R, tag="lnrow4", bufs=1, name=f"rs2{uid}")
                    nc.scalar.mul(rs_r2[:, :], rs_r[:, :], 1.0)
                    muB = ps4.tile([P, R], F32, tag="pmu", name=f"muBp{uid}")
                    nc.tensor.matmul(muB[:, :], onesr[:, :], mu_r[:, :])
                    rsBp = ps4.tile([P, R], F32, tag="psq", name=f"rsBp{uid}")
                    nc.tensor.matmul(rsBp[:, :], onesr[:, :], rs_r2[:, :])
                    muBb = sqp.tile([P, R], BF16, tag="muBb", name=f"muBb{uid}", bufs=1)
                    nc.scalar.activation(muBb[:, :], muB[:, :], AF.Copy)
                    rsBb = sqp.tile([P, R], BF16, tag="rsBb", name=f"rsBb{uid}", bufs=1)
                    nc.vector.tensor_copy(rsBb[:, :], rsBp[:, :])
                    for ot in range(NT):
                        t1 = sqp.tile([P, R], BF16, tag="tt", name=f"t1{uid}_{ot}", bufs=4)
                        nc.vector.tensor_sub(t1[:, :], resT[:, ot, :], muBb[:, :])
                        t2 = sqp.tile([P, R], BF16, tag="tt", name=f"t2{uid}_{ot}", bufs=4)
                        if ot % 2 == 0 or (not store and ot < 2):
                            nc.vector.tensor_mul(t2[:, :], t1[:, :], rsBb[:, :])
                        else:
                            nc.gpsimd.tensor_mul(t2[:, :], t1[:, :], rsBb[:, :])
                        if store:
                            oT = outp.tile([P, R], BF16, tag="oT", name=f"oT{uid}_{ot}")
                            nc.scalar.activation(
                                oT[:, :], t2[:, :], AF.Identity,
                                bias=prm[:, I_B, ot, :], scale=prm[:, I_G, ot, :])
                            nc.sync.dma_start(out=out_d.ap()[:, ot, :], in_=oT[:, :])
                        elif ot < 2:
                            # first tiles gate the next attention's scores:
                            # keep their whole chain on the fast Vector path
                            nc.vector.tensor_scalar(
                                dst[ot][:, :], t2[:, :], prm[:, I_G, ot, :],
                                prm[:, I_B, ot, :], op0=ALU.mult, op1=ALU.add)
                        elif ot % 2 == 0:
                            nc.gpsimd.tensor_scalar(
                                dst[ot][:, :], t2[:, :], prm[:, I_G, ot, :],
                                prm[:, I_B, ot, :], op0=ALU.mult, op1=ALU.add)
                        else:
                            nc.vector.tensor_scalar(
                                dst[ot][:, :], t2[:, :], prm[:, I_G, ot, :],
                                prm[:, I_B, ot, :], op0=ALU.mult, op1=ALU.add)

            # ================== phase 1: k0 + q0 only ========================
            with tc.tile_pool(name="wqp", bufs=3) as wqp, \
                 tc.tile_pool(name="psA", bufs=2, space="PSUM") as psA:
                # PE p-state warm-up: ~4us of dummy matmuls during the
                # input-DMA wait so k0/q0 don't run at cold-pipeline rate
                with tc.tile_pool(name="wup", bufs=1, space="PSUM") as wup:
                    wu = wup.tile([1, R], F32, tag="wu", name="wu")
                    for _ in range(10):
                        nc.tensor.matmul(wu[:, :], onesb[:, :], wuM[:, :],
                                         start=True, stop=True)
                k_unit(x8, 0, psA, "a")
                q_unit(0, psA, wqp, pre=wq0t)

                # ===== phase 2: self-attn; k1/q1 as prefills; dec v-half-0
                # feeds pair 0's PV steps via the per-step drain schedule ==
                pre = [lambda: k_unit(x8, 1, psA, "a"),
                       lambda: q_unit(1, psA, wqp, pre=wq1t)]
                fillsA = [lambda kb=kb: v_unit(x8, kb, 0, psA, "a") for kb in range(NT)]
                for ot in (2, 3):
                    fillsA.append(lambda ot=ot: k_unit(x8, ot, psA, "a"))
                    fillsA.append(lambda ot=ot: q_unit(ot, psA, wqp))
                fillsA.append(lambda: v_unit(x8, 0, 1, psA, "a"))
                fillsA.append(lambda: v_unit(x8, 1, 1, psA, "a"))
                for ot in (4, 5):
                    fillsA.append(lambda ot=ot: k_unit(x8, ot, psA, "a"))
                    fillsA.append(lambda ot=ot: q_unit(ot, psA, wqp))
                for kb in range(2, NT):
                    fillsA.append(lambda kb=kb: v_unit(x8, kb, 1, psA, "a"))
                for ot in (6, 7):
                    fillsA.append(lambda ot=ot: k_unit(x8, ot, psA, "a"))
                    fillsA.append(lambda ot=ot: q_unit(ot, psA, wqp))
                # attnA's last two pairs have PE slack (Scalar-bound):
                # start the encoder k0/k1 there for attnB's first scores
                fillsA.append(lambda: k_unit(x8e, 0, psA, "b"))
                fillsA.append(lambda: k_unit(x8e, 1, psA, "b"))
                # fills: [v0..v7, k2,q2,k3,q3, v1_0,v1_1, k4,q4,k5,q5,
                #         v1_2..v1_7, k6,q6,k7,q7, ke0,ke1] = 30
                schedA = [[2, 4, 6, 8],        # v0..v7 feed pair-0 PV steps
                          [9, 10, 11, 12],     # k2,q2,k3,q3
                          [13, 14, 15, 16],    # v1_0,v1_1,k4,q4
                          [17, 18, 19, 20],    # v1_2,v1_3,k5,q5
                          [21, 22, 23, 24],    # v1_4..v1_7 (feed pair-4 PV)
                          [25, 26, 27, 28],    # k6,q6,k7,q7
                          [29, 30, 30, 30],    # ke0,ke1
                          [30] * 4]
                attention(q1b, I_MT, fillsA, schedA, "A", p8A, prefills=pre)

            # ========= phase 3: out-proj + LN1 (+ enc v-half-0 interleaved:
            # keeps the PE stream dense through the stats chain) ==============
            with tc.tile_pool(name="psV", bufs=1, space="PSUM") as psV:
                fillsL = [lambda kb=kb: v_unit(x8e, kb, 0, psV, "b")
                          for kb in range(NT)]
                proj_ln(q1b, slfb, fillsL, "A", hold=4)

            # ==================== phase 4: cross-attention ====================
            with tc.tile_pool(name="psB", bufs=2, space="PSUM") as psB:
                def mk_pAa(ot):
                    def f():
                        pp = psB.tile([P, R], F32, tag="pk", name=f"ppA{ot}")
                        for i in range(2):
                            nc.tensor.matmul(
                                pp[:, :], wdr[:, ot, 2 * i:2 * i + 2, :],
                                ctxn[:, 2 * i:2 * i + 2, :],
                                start=(i == 0), stop=(i == 1), perf_mode=DR)
                        pt = sqpB.tile([P, R], BF16, tag="pt", name=f"pt{ot}", bufs=2)
                        nc.vector.tensor_scalar(
                            pt[:, :], pp[:, :], INV, prm[:, I_BDE, ot, :],
                            op0=ALU.mult, op1=ALU.add)
                        # pre-fold the residual so the LN2 evict is one STT
                        nc.vector.tensor_add(
                            partialA[:, ot, :], pt[:, :], slfb[ot][:, :])
                    return f

                def mk_pAb(ot):
                    def f():
                        pp = psB.tile([P, R], F32, tag="pk", name=f"ppB{ot}")
                        nc.tensor.matmul(
                            pp[:, :], wdr[:, ot, 4:6, :], ctxn[:, 4:6, :],
                            start=True, stop=True, perf_mode=DR)
                        nc.vector.scalar_tensor_tensor(
                            partialA[:, ot, :], pp[:, :], INV,
                            partialA[:, ot, :], op0=ALU.mult, op1=ALU.add)
                    return f

                with tc.tile_pool(name="sqpB", bufs=2) as sqpB:
                    ke = [lambda ot=ot: k_unit(x8e, ot, psB, "b") for ot in range(2, NT)]
                    ve = [lambda kb=kb: v_unit(x8e, kb, 1, psB, "b") for kb in range(NT)]
                    # interleave the heavy ke units with light ve units and
                    # keep attnB's first pairs (gated on the slfb trickle)
                    # nearly fill-free so the exp stream ramps immediately.
                    # Readiness: ke_j before scores(B,j) at (j-1,k3); ve1_kb
                    # before PV(B,4,k=kb//2); pAa >= pair 4; pAb >= pair 6.
                    fillsB = [ke[0], ke[1], ke[2], ve[0], ke[3], ve[1],
                              ke[4], ve[2], ke[5], ve[3], ve[4], ve[5],
                              ve[6], ve[7]]
                    fillsB += [mk_pAa(ot) for ot in range(NT)]
                    fillsB += [mk_pAb(ot) for ot in range(NT)]
                    schedB = [[0, 0, 1, 1],      # ke2
                              [2, 2, 3, 3],      # ke3,ke4
                              [4, 5, 6, 7],      # ve1_0,ke5,ve1_1,ke6
                              [8, 9, 10, 11],    # ve1_2,ke7,ve1_3,ve1_4
                              [12, 13, 14, 15],  # ve1_5..7, pAa0
                              [16, 17, 18, 19],  # pAa1..4
                              [20, 21, 22, 23],  # pAa5..7, pAb0
                              [25, 27, 29, 30]]  # pAb1..7
                    attention(slfb, I_MS, fillsB, schedB, "B", p8B)

            # ============ phase 5: out-proj + LN2 + store ====================
            proj_ln(slfb, None, [], "B", partial=partialA, store=True)
            rcp.release()

    nc.compile()
    return nc


_NC = None

_F8NP = ml_dtypes.float8_e4m3
_BFNP = ml_dtypes.bfloat16
_ONESR = np.ones((1, P), np.float32)


def make_in_maps(encoder_states, decoder_inputs, src_attention_mask,
                 tgt_attention_mask, Wq, bq, Wk, bk, Wv, bv, Wd, bd, ln_g, ln_b):
    f = np.float32

    def wtile(w, dt, scale=1.0):  # [o,i] -> W.T chunks [ot, p_i, it, p_o]
        a = (np.asarray(w, f).T * scale).reshape(NT, P, NT, P)
        return np.ascontiguousarray(a.transpose(2, 1, 0, 3)).astype(dt)

    def atile(x, dt):  # [t,i] -> x.T tiled [p, it, t]
        return np.ascontiguousarray(
            np.asarray(x, f).T.reshape(NT, P, -1).transpose(1, 0, 2)).astype(dt)

    col = lambda x: np.asarray(x, f).reshape(NT, P).T.reshape(P, NT, 1)

    wq_t = wtile(Wq, _BFNP)
    # wd resident layout [p_i, ot, it, p_o] (contiguous DMA)
    wd_t = np.ascontiguousarray(
        wtile(Wd, np.float32, WS).transpose(1, 0, 2, 3)).astype(_F8NP)
    # wk resident layout [p_i, ot, it, p_o]
    wk_t = np.ascontiguousarray(
        wtile(Wk, np.float32, WS).transpose(1, 0, 2, 3)).astype(_F8NP)
    # wv v-direct layout [p_i, hf, it, o-within-half] (contiguous DMA halves)
    wv_t = np.ascontiguousarray(
        (np.asarray(Wv, f).T * WS).reshape(NT, P, 2, R).transpose(1, 2, 0, 3)
    ).astype(_F8NP)
    bde = np.asarray(bd, f) + np.asarray(bv, f) @ np.asarray(Wd, f).T
    bq_, bde_ = col(bq), col(bde)
    g_, b_ = col(ln_g), col(ln_b)

    dec8_b = [atile(decoder_inputs[b], _F8NP) for b in range(B)]
    enc8_b = [atile(encoder_states[b], _F8NP) for b in range(B)]
    prm_b = []
    for b in range(B):
        mt_ = col(tgt_attention_mask[b, 0, 0, :])
        ms_ = col(src_attention_mask[b, 0, 0, :])
        prm_b.append(np.ascontiguousarray(
            np.stack([bq_, bde_, g_, b_, mt_, ms_], axis=1)))  # [P, 6, NT, 1]

    in_maps = []
    for c in range(8):
        b, half = c // 2, c % 2
        in_maps.append({
            "dec8": dec8_b[b],
            "enc8": enc8_b[b],
            "dqb": atile(decoder_inputs[b][half * R:(half + 1) * R], _BFNP),
            "wq": wq_t, "wk": wk_t, "wv": wv_t, "wd": wd_t,
            "prm": prm_b[b],
            "onesr": _ONESR,
        })
    return in_maps


def kernel(**inputs):
    global _NC
    if _NC is None:
        _NC = build_kernel()
    nc = _NC
    in_maps = make_in_maps(**inputs)
    res = run_bass_kernel_spmd(nc, in_maps, core_ids=list(range(8)))
    out = np.empty((B, T, H), np.float32)
    for c in range(8):
        b, half = c // 2, c % 2
        buf = np.asarray(res.results[c]["out"]).astype(np.float32)  # [p, ot, t]
        out[b, half * R:(half + 1) * R, :] = (
            buf.transpose(2, 1, 0).reshape(R, H))
    return out
